# revision 1
# baseline (speedup 1.0000x reference)
"""DP-means (2-iteration early-stop) Trainium2 kernel — feature-sharded.

Each of the 8 cores owns a 128-feature slice of all 65536 points:
  xh  [N, 128]  bf16 natural slice   (streamed 3x: colsum, S_far, segsums)
  xth [128, N]  bf16 transposed      (streamed 2x: it0/it1 dot stationaries)
  xtl [128, N]  bf16 residual transp (RESIDENT in SBUF: lo-dots, colsum-lo,
                                      S_far-lo via masked reduce)
  x2pm [128, 512] fp32 host-exact point norms, pm layout (replicated)

Cross-core communication is 2 AllReduces of per-point dot partials
(it0: 256KB, it1: 512KB) with the mu-norm partial scalars folded into the
payload. colsum / S_far / segment sums are core-local (feature sharding).
Distances use bf16 hi/lo split products (xh*mh + xh*ml + xl*mh) so the
far/argmin decisions match fp32; final segment sums are bf16-hi only.
Host does only O(N+D) work: input prep, x2, epilogue divisions, guards.

pm layout: point i = t*128 + p  ->  pm[p, t], t in [0, 512).
"""
import sys

sys.path.insert(0, "/opt/trn_rl_repo")

import numpy as np

import concourse.bass as bass
import concourse.bacc as bacc
import concourse.tile as tile
from concourse import mybir
from concourse import bass_utils

F32 = mybir.dt.float32
BF16 = mybir.dt.bfloat16
Alu = mybir.AluOpType
Act = mybir.ActivationFunctionType

N_FULL, D, K_MAX = 65536, 1024, 64
NCORES = 8
LAMBDA = 1000.0
MAX_ITER = 50
OBJ_TOL = 1e-3
P = 128
FC = D // NCORES      # 128 features per core
NT = N_FULL // P      # 512 pm tiles
NSC = 8               # output scalar slots


def build_kernel(n_total: int = N_FULL, ncores: int = NCORES, reps: int = 1, upto: int = 5, fake_coll: bool = False, no_masked: bool = False):
    NS = n_total
    nt = NS // P
    nchunk = NS // 4096          # xth/xtl processing chunks (4096 pts)
    ngrp = NS // 2048            # xh natural stream groups (16 tiles each)

    nc = bacc.Bacc("TRN2", target_bir_lowering=False, debug=False,
                   num_devices=ncores)
    xh = nc.dram_tensor("xh", [P, NS], BF16, kind="ExternalInput")
    xth = nc.dram_tensor("xth", [FC, NS], BF16, kind="ExternalInput")
    xtl = nc.dram_tensor("xtl", [FC, NS], BF16, kind="ExternalInput")
    xlp = nc.dram_tensor("xlp", [P, NS], BF16, kind="ExternalInput")
    x2pm_in = nc.dram_tensor("x2pm", [P, nt], F32, kind="ExternalInput")
    o_out = nc.dram_tensor("o_out", [1, 5 * P + NSC], F32, kind="ExternalOutput")

    rg = [list(range(ncores))]

    with tile.TileContext(nc) as tc:
        with (
            tc.tile_pool(name="persist", bufs=1) as pp,
            tc.tile_pool(name="xhs", bufs=4) as xp,
            tc.tile_pool(name="xts", bufs=2) as tp,
            tc.tile_pool(name="scratch", bufs=1) as scp,
            tc.tile_pool(name="bc", bufs=2) as bcp,
            tc.tile_pool(name="psum", bufs=1, space="PSUM") as psp,
            tc.tile_pool(name="dram", bufs=1, space="DRAM") as dp,
        ):
            def _body():
                onesf = pp.tile([P, 1], F32)
                nc.gpsimd.memset(onesf[:], 1.0)
                onesb = pp.tile([P, 1], BF16)
                nc.gpsimd.memset(onesb[:], 1.0)
                ones_row = pp.tile([1, P], F32)
                nc.gpsimd.memset(ones_row[:], 1.0)
                one11 = pp.tile([1, 1], F32)
                nc.gpsimd.memset(one11[:], 1.0)

                def mm_bcast(dst, s11, stag):
                    rowt = scp.tile([1, P], F32, tag="row" + stag)
                    nc.vector.tensor_scalar(rowt[:], ones_row[:], s11[:], None,
                                            op0=Alu.mult)
                    ps = psp.tile([P, 1], F32, tag="bc11", bufs=2)
                    nc.tensor.matmul(ps[:], rowt[:], one11[:], start=True, stop=True)
                    nc.vector.tensor_copy(dst[:], ps[:])

                # ---------- resident xtl (16MB) ----------
                xtl_sb = pp.tile([P, NS], BF16)
                for q in range(4):
                    nc.sync.dma_start(xtl_sb[:, q * (NS // 4):(q + 1) * (NS // 4)],
                                      xtl[:, q * (NS // 4):(q + 1) * (NS // 4)])

                # persistent pm tensors
                x2pm = pp.tile([P, nt], F32)
                nc.sync.dma_start(x2pm[:], x2pm_in[:, :])
                d0pm = pp.tile([P, nt], F32)
                d1pm = pp.tile([P, nt], F32)
                far0 = pp.tile([P, nt], BF16)
                z1 = pp.tile([P, nt], BF16)
                far1 = pp.tile([P, nt], BF16)
                nf1 = pp.tile([P, nt], BF16)
                A3 = pp.tile([P, 3 * nt], BF16)
                tmpa = scp.tile([P, nt], F32, tag="tmpa")
                tmpb = scp.tile([P, nt], F32, tag="tmpb")
                dmin = pp.tile([P, nt], F32)

                # scalars / vectors
                cslo8 = pp.tile([P, 8], F32)
                colsum = pp.tile([P, 1], F32)
                cs_hi = pp.tile([P, 1], F32)
                cs_lo = pp.tile([P, 1], F32)
                mu0 = pp.tile([P, 1], F32)
                mu02b = pp.tile([P, 2], BF16)
                mh0f = pp.tile([P, 1], F32)
                ml0f = pp.tile([P, 1], F32)
                sfar = pp.tile([P, 1], F32)
                c1 = pp.tile([P, 1], F32)
                mu0p = pp.tile([P, 1], F32)
                mu14b = pp.tile([P, 4], BF16)
                mu12lob = pp.tile([P, 2], BF16)
                m02b = pp.tile([P, 1], F32)
                m02pb = pp.tile([P, 1], F32)
                mc1b = pp.tile([P, 1], F32)
                rnfb = pp.tile([P, 1], F32)
                rnnb = pp.tile([P, 1], F32)
                nfar_p = pp.tile([P, 1], F32)

                # DRAM
                b1i = dp.tile([1, NS + 16], F32)
                b1o = dp.tile([1, NS + 16], F32)
                b2i = dp.tile([1, 2 * NS + 16], F32)
                b2o = dp.tile([1, 2 * NS + 16], F32)
                sdram = dp.tile([1, 16], F32)

                def bcast(dst, buf, off):
                    src = buf[:, off:off + 1].rearrange("o u -> (o u)")
                    rep = bass.AP(src.tensor, src.offset, [[0, P]] + src.ap)
                    nc.scalar.dma_start(dst[:], rep)

                def psum11(col_f32, stag):
                    ps = psp.tile([1, 1], F32, tag="t11", bufs=1)
                    nc.tensor.matmul(ps[:], col_f32[:], onesf[:], start=True, stop=True)
                    s11 = scp.tile([1, 1], F32, tag="s11" + stag)
                    nc.vector.tensor_copy(s11[:], ps[:])
                    return s11

                # ---------- P1: colsum ----------
                # lo: free-reduce resident xtl on ACT (Copy + accum_out)
                for c in range(8):
                    sl = slice(c * (NS // 8), (c + 1) * (NS // 8))
                    nc.vector.tensor_reduce(cslo8[:, c:c + 1], xtl_sb[:, sl],
                                            axis=mybir.AxisListType.X, op=Alu.add)
                nc.vector.tensor_reduce(cs_lo[:], cslo8[:],
                                        axis=mybir.AxisListType.X, op=Alu.add)
                # hi: stream xh; stationary = xg tile, moving = ones -> [128,1] col
                cs_ps = psp.tile([P, 1], F32, tag="sums")
                for g in range(ngrp):
                    xg = xp.tile([P, 16 * FC], BF16, tag="xg")
                    nc.sync.dma_start(xg[:], xh[:, g * 2048:(g + 1) * 2048])
                    for k in range(16):
                        nc.tensor.matmul(cs_ps[:], xg[:, k * FC:(k + 1) * FC],
                                         onesb[:],
                                         start=(g == 0 and k == 0),
                                         stop=(g == ngrp - 1 and k == 15))
                nc.vector.tensor_copy(cs_hi[:], cs_ps[:])
                nc.vector.tensor_tensor(colsum[:], cs_hi[:], cs_lo[:], op=Alu.add)
                # mu0 slice, hi/lo split, m02 partial
                nc.vector.tensor_scalar(mu0[:], colsum[:], 1.0 / n_total, None,
                                        op0=Alu.mult)
                nc.vector.tensor_copy(mu02b[:, 0:1], mu0[:])
                nc.vector.tensor_copy(mh0f[:], mu02b[:, 0:1])
                nc.vector.tensor_tensor(ml0f[:], mu0[:], mh0f[:], op=Alu.subtract)
                nc.vector.tensor_copy(mu02b[:, 1:2], ml0f[:])
                sq = scp.tile([P, 1], F32, tag="sq")
                nc.vector.tensor_tensor(sq[:], mu0[:], mu0[:], op=Alu.mult)
                m02s = psum11(sq, 'm02')
                nc.scalar.dma_start(b1i[:, NS:NS + 1], m02s[:])

                if upto < 2:
                    return
                # ---------- P2: it0 dots ----------
                for c in range(nchunk):
                    csl = slice(c * 4096, (c + 1) * 4096)
                    xtg = tp.tile([P, 4096], BF16, tag="xtg")
                    nc.sync.dma_start(xtg[:], xth[:, csl])
                    hi_ps = psp.tile([P, 128], F32, tag="hi", bufs=2)
                    lo_ps = psp.tile([P, 64], F32, tag="lo", bufs=2)
                    for k in range(32):
                        t = c * 32 + k
                        nc.tensor.matmul(hi_ps[:, 2 * k:2 * k + 2],
                                         xtg[:, k * P:(k + 1) * P],
                                         mu02b[:], start=True, stop=True)
                        nc.tensor.matmul(lo_ps[:, k:k + 1],
                                         xtl_sb[:, t * P:(t + 1) * P],
                                         mu02b[:, 0:1], start=True, stop=True)
                    hv = hi_ps[:, 0:64].rearrange("p (k two) -> p k two", two=2)
                    t32 = scp.tile([P, 32], F32, tag="t32")
                    nc.vector.tensor_copy(t32[:], hv[:, :, 0])
                    nc.vector.tensor_tensor(t32[:], t32[:], hv[:, :, 1], op=Alu.add)
                    nc.vector.tensor_tensor(d0pm[:, c * 32:(c + 1) * 32],
                                            t32[:], lo_ps[:, 0:32], op=Alu.add)
                nc.scalar.dma_start(
                    b1i[:, 0:NS].rearrange("o (p t) -> (o p) t", t=nt), d0pm[:])
                if fake_coll:
                    nc.sync.dma_start(b1o[:], b1i[:])
                else:
                    nc.gpsimd.collective_compute(
                        "AllReduce", Alu.add, replica_groups=rg,
                        ins=[b1i.opt()], outs=[b1o.opt()])
                # ---------- far0, S_far ----------
                nc.scalar.dma_start(
                    d0pm[:], b1o[:, 0:NS].rearrange("o (p t) -> (o p) t", t=nt))
                bcast(m02b, b1o, NS)
                nc.vector.tensor_scalar(tmpa[:], d0pm[:], -2.0, None, op0=Alu.mult)
                nc.vector.tensor_tensor(tmpb[:], tmpa[:], x2pm[:], op=Alu.add)
                nc.vector.tensor_scalar(tmpa[:], tmpb[:], m02b[:], None, op0=Alu.add)
                nc.vector.tensor_scalar(far0[:], tmpa[:], LAMBDA, None, op0=Alu.is_gt)
                # nfar, 1/nfar, 1/(n-nfar)
                nc.vector.tensor_reduce(nfar_p[:], far0[:],
                                        axis=mybir.AxisListType.X, op=Alu.add)
                nf11 = psum11(nfar_p, 'nf')
                rnf11 = scp.tile([1, 1], F32, tag="rnf")
                nc.vector.reciprocal(rnf11[:], nf11[:])
                nn11 = scp.tile([1, 1], F32, tag="nn")
                nc.vector.tensor_scalar(nn11[:], nf11[:], -1.0, float(n_total),
                                        op0=Alu.mult, op1=Alu.add)
                rnn11 = scp.tile([1, 1], F32, tag="rnn")
                nc.vector.reciprocal(rnn11[:], nn11[:])
                mm_bcast(rnfb, rnf11, "rnf")
                mm_bcast(rnnb, rnn11, "rnn")
                # sfx2_0 = sum(far0 * x2)
                nc.vector.tensor_copy(tmpa[:], far0[:])
                nc.vector.tensor_tensor(tmpb[:], tmpa[:], x2pm[:], op=Alu.mult)
                sfx0_p = scp.tile([P, 1], F32, tag="sfx0")
                nc.vector.tensor_reduce(sfx0_p[:], tmpb[:],
                                        axis=mybir.AxisListType.X, op=Alu.add)
                if upto < 3:
                    return
                # S_far hi+lo: stream xh and xlp; stationary = data tiles,
                # moving = far0 col -> psum [128, 2] (hi col 0, lo col 1)
                sf_ps = psp.tile([P, 2], F32, tag="sums")
                for g in range(ngrp):
                    xg = xp.tile([P, 16 * FC], BF16, tag="xg")
                    nc.sync.dma_start(xg[:], xh[:, g * 2048:(g + 1) * 2048])
                    xlg = xp.tile([P, 16 * FC], BF16, tag="xlg")
                    nc.sync.dma_start(xlg[:], xlp[:, g * 2048:(g + 1) * 2048])
                    for k in range(16):
                        t = g * 16 + k
                        nc.tensor.matmul(sf_ps[:, 0:1], xg[:, k * FC:(k + 1) * FC],
                                         far0[:, t:t + 1],
                                         start=(g == 0 and k == 0),
                                         stop=(g == ngrp - 1 and k == 15))
                        nc.tensor.matmul(sf_ps[:, 1:2], xlg[:, k * FC:(k + 1) * FC],
                                         far0[:, t:t + 1],
                                         start=False, skip_group_check=True,
                                         stop=(g == ngrp - 1 and k == 15))
                sf2 = scp.tile([P, 2], F32, tag="sf2")
                nc.vector.tensor_copy(sf2[:], sf_ps[:])
                nc.vector.tensor_tensor(sfar[:], sf2[:, 0:1], sf2[:, 1:2],
                                        op=Alu.add)
                # c1, mu0p slices + norm partials
                nc.vector.tensor_scalar(c1[:], sfar[:], rnfb[:], None, op0=Alu.mult)
                t1 = scp.tile([P, 1], F32, tag="t1")
                nc.vector.tensor_tensor(t1[:], colsum[:], sfar[:], op=Alu.subtract)
                nc.vector.tensor_scalar(mu0p[:], t1[:], rnnb[:], None, op0=Alu.mult)
                # hi/lo splits: mu14b = [mh0p, ml0p, mhc1, mlc1]; mu12lob = [mh0p, mhc1]
                mhf = scp.tile([P, 1], F32, tag="mhf")
                for j, vec in ((0, mu0p), (1, c1)):
                    nc.vector.tensor_copy(mu14b[:, 2 * j:2 * j + 1], vec[:])
                    nc.vector.tensor_copy(mhf[:], mu14b[:, 2 * j:2 * j + 1])
                    nc.vector.tensor_tensor(t1[:], vec[:], mhf[:], op=Alu.subtract)
                    nc.vector.tensor_copy(mu14b[:, 2 * j + 1:2 * j + 2], t1[:])
                    nc.vector.tensor_copy(mu12lob[:, j:j + 1], vec[:])
                    nc.vector.tensor_tensor(sq[:], vec[:], vec[:], op=Alu.mult)
                    s = psum11(sq, 'nrm%d' % j)
                    nc.scalar.dma_start(b2i[:, 2 * NS + j:2 * NS + j + 1], s[:])

                if upto < 4:
                    return
                # ---------- P3: it1 dots ----------
                for c in range(nchunk):
                    csl = slice(c * 4096, (c + 1) * 4096)
                    xtg = tp.tile([P, 4096], BF16, tag="xtg")
                    nc.sync.dma_start(xtg[:], xth[:, csl])
                    hi_ps = psp.tile([P, 128], F32, tag="hi", bufs=2)
                    lo_ps = psp.tile([P, 64], F32, tag="lo", bufs=2)
                    for k in range(32):
                        t = c * 32 + k
                        nc.tensor.matmul(hi_ps[:, 4 * k:4 * k + 4],
                                         xtg[:, k * P:(k + 1) * P],
                                         mu14b[:], start=True, stop=True)
                        nc.tensor.matmul(lo_ps[:, 2 * k:2 * k + 2],
                                         xtl_sb[:, t * P:(t + 1) * P],
                                         mu12lob[:], start=True, stop=True)
                    hv = hi_ps[:].rearrange("p (k four) -> p k four", four=4)
                    lv = lo_ps[:].rearrange("p (k two) -> p k two", two=2)
                    t32 = scp.tile([P, 32], F32, tag="t32")
                    nc.vector.tensor_copy(t32[:], hv[:, :, 0])
                    nc.vector.tensor_tensor(t32[:], t32[:], hv[:, :, 1], op=Alu.add)
                    nc.vector.tensor_tensor(d0pm[:, c * 32:(c + 1) * 32],
                                            t32[:], lv[:, :, 0], op=Alu.add)
                    nc.vector.tensor_copy(t32[:], hv[:, :, 2])
                    nc.vector.tensor_tensor(t32[:], t32[:], hv[:, :, 3], op=Alu.add)
                    nc.vector.tensor_tensor(d1pm[:, c * 32:(c + 1) * 32],
                                            t32[:], lv[:, :, 1], op=Alu.add)
                nc.scalar.dma_start(
                    b2i[:, 0:NS].rearrange("o (p t) -> (o p) t", t=nt), d0pm[:])
                nc.scalar.dma_start(
                    b2i[:, NS:2 * NS].rearrange("o (p t) -> (o p) t", t=nt), d1pm[:])
                if fake_coll:
                    nc.sync.dma_start(b2o[:], b2i[:])
                else:
                    nc.gpsimd.collective_compute(
                        "AllReduce", Alu.add, replica_groups=rg,
                        ins=[b2i.opt()], outs=[b2o.opt()])
                # ---------- masks, stats, segsums ----------
                nc.scalar.dma_start(
                    d0pm[:], b2o[:, 0:NS].rearrange("o (p t) -> (o p) t", t=nt))
                nc.scalar.dma_start(
                    d1pm[:], b2o[:, NS:2 * NS].rearrange("o (p t) -> (o p) t", t=nt))
                bcast(m02pb, b2o, 2 * NS)
                bcast(mc1b, b2o, 2 * NS + 1)
                dist0 = scp.tile([P, nt], F32, tag="dist0")
                dist1 = scp.tile([P, nt], F32, tag="dist1")
                nc.vector.tensor_scalar(tmpa[:], d0pm[:], -2.0, None, op0=Alu.mult)
                nc.vector.tensor_tensor(tmpb[:], tmpa[:], x2pm[:], op=Alu.add)
                nc.vector.tensor_scalar(dist0[:], tmpb[:], m02pb[:], None, op0=Alu.add)
                nc.vector.tensor_scalar(tmpa[:], d1pm[:], -2.0, None, op0=Alu.mult)
                nc.vector.tensor_tensor(tmpb[:], tmpa[:], x2pm[:], op=Alu.add)
                nc.vector.tensor_scalar(dist1[:], tmpb[:], mc1b[:], None, op0=Alu.add)
                nc.vector.tensor_tensor(tmpa[:], dist1[:], dist0[:], op=Alu.is_lt)
                nc.vector.tensor_copy(z1[:], tmpa[:])
                nc.vector.tensor_tensor(dmin[:], dist0[:], dist1[:], op=Alu.min)
                nc.vector.tensor_scalar(far1[:], dmin[:], LAMBDA, None, op0=Alu.is_gt)
                nc.vector.tensor_scalar(nf1[:], far1[:], -1.0, 1.0,
                                        op0=Alu.mult, op1=Alu.add)
                # A3 interleaved [A0 A1 far1]
                A3v = A3[:].rearrange("p (t k) -> p t k", k=3)
                z1c = scp.tile([P, nt], BF16, tag="z1c")
                nc.vector.tensor_scalar(z1c[:], z1[:], -1.0, 1.0,
                                        op0=Alu.mult, op1=Alu.add)
                nc.vector.tensor_tensor(A3v[:, :, 0], z1c[:], nf1[:], op=Alu.mult)
                nc.vector.tensor_tensor(A3v[:, :, 1], z1[:], nf1[:], op=Alu.mult)
                nc.vector.tensor_copy(A3v[:, :, 2], far1[:])
                # stats: counts, snfdmin, sfx2_1
                pack8 = pp.tile([P, NSC], F32)
                nc.vector.tensor_copy(pack8[:, 0:1], nfar_p[:])
                nc.vector.tensor_copy(pack8[:, 5:6], sfx0_p[:])
                for j in range(3):
                    nc.vector.tensor_reduce(pack8[:, 1 + j:2 + j], A3v[:, :, j],
                                            axis=mybir.AxisListType.X, op=Alu.add)
                nc.vector.tensor_copy(tmpa[:], nf1[:])
                nc.vector.tensor_tensor(tmpb[:], dmin[:], tmpa[:], op=Alu.mult)
                nc.vector.tensor_reduce(pack8[:, 4:5], tmpb[:],
                                        axis=mybir.AxisListType.X, op=Alu.add)
                nc.vector.tensor_copy(tmpa[:], far1[:])
                nc.vector.tensor_tensor(tmpb[:], tmpa[:], x2pm[:], op=Alu.mult)
                nc.vector.tensor_reduce(pack8[:, 6:7], tmpb[:],
                                        axis=mybir.AxisListType.X, op=Alu.add)
                nc.vector.tensor_copy(pack8[:, 7:8], pack8[:, 6:7])
                if upto < 5:
                    return
                # segment sums: stream xh; stationary = A3 3 cols per tile
                s3_ps = psp.tile([P, 3], F32, tag="sums")
                for g in range(ngrp):
                    xg = xp.tile([P, 16 * FC], BF16, tag="xg")
                    nc.sync.dma_start(xg[:], xh[:, g * 2048:(g + 1) * 2048])
                    for k in range(16):
                        t = g * 16 + k
                        nc.tensor.matmul(s3_ps[:], xg[:, k * FC:(k + 1) * FC],
                                         A3[:, 3 * t:3 * t + 3],
                                         start=(g == 0 and k == 0),
                                         stop=(g == ngrp - 1 and k == 15))
                s3_sb = scp.tile([P, 3], F32, tag="s3sb")
                nc.vector.tensor_copy(s3_sb[:], s3_ps[:])
                # ---------- outputs ----------
                nc.sync.dma_start(
                    o_out[:, 0:P].rearrange("o (p u) -> (o p) u", u=1), colsum[:])
                nc.sync.dma_start(
                    o_out[:, P:2 * P].rearrange("o (p u) -> (o p) u", u=1), sfar[:])
                nc.sync.dma_start(
                    o_out[:, 2 * P:5 * P].rearrange("o (p k) -> (o p) k", k=3),
                    s3_sb[:])
                sc_ps = psp.tile([NSC, 1], F32, tag="t11")
                nc.tensor.matmul(sc_ps[:], pack8[:], onesf[:], start=True, stop=True)
                sc81 = scp.tile([NSC, 1], F32, tag="sc81")
                nc.vector.tensor_copy(sc81[:], sc_ps[:])
                nc.scalar.dma_start(
                    o_out[:, 5 * P:5 * P + NSC]
                    .rearrange("o (r u) -> (o r) u", u=1), sc81[:])

            for _rep in range(reps):
                _body()

    nc.compile()
    return nc


import ml_dtypes


def prep_core(X, c):
    """Per-core inputs from full X [N, D] fp32."""
    xs = np.ascontiguousarray(X[:, c * FC:(c + 1) * FC])
    xhn = xs.astype(ml_dtypes.bfloat16)
    xl32 = xs - xhn.astype(np.float32)
    xthn = np.ascontiguousarray(xhn.T)
    xtln = np.ascontiguousarray(xl32.T.astype(ml_dtypes.bfloat16))
    n = xs.shape[0]
    xhp = np.ascontiguousarray(
        xhn.reshape(n // P, P, FC).transpose(1, 0, 2).reshape(P, n))
    xln = xl32.astype(ml_dtypes.bfloat16)
    xlpp = np.ascontiguousarray(
        xln.reshape(n // P, P, FC).transpose(1, 0, 2).reshape(P, n))
    return {"xh": xhp, "xth": xthn, "xtl": xtln, "xlp": xlpp}


_NC_CACHE = {}


def _get_nc(n_total, ncores):
    key = (n_total, ncores)
    if key not in _NC_CACHE:
        _NC_CACHE[key] = build_kernel(n_total, ncores)
    return _NC_CACHE[key]


def _dpmeans_numpy_fallback(X):
    n, d = X.shape
    mu = np.zeros((K_MAX, d), np.float32)
    mu[0] = X.mean(axis=0)
    K = 1
    x2 = np.sum(X * X, axis=1)
    prev_obj = 0.0
    for it in range(MAX_ITER):
        m2 = np.sum(mu * mu, axis=1)
        dist = x2[:, None] - 2.0 * (X @ mu.T) + m2[None, :]
        dist[:, K:] = 1e30
        dmin = dist.min(axis=1)
        z = dist.argmin(axis=1)
        far = dmin > LAMBDA
        create = bool(far.any()) and K < K_MAX
        Kc = min(K, K_MAX - 1)
        nfar = float(far.sum())
        new_center = (far.astype(np.float32) @ X) / max(nfar, 1.0)
        mu_c = mu.copy() if create else mu
        if create:
            mu_c[Kc] = new_center
            z = np.where(far, Kc, z)
            new_col = x2 - 2.0 * (X @ new_center) + float(new_center @ new_center)
            dvals = np.where(far, new_col, dmin)
        else:
            dvals = dmin
        K = K + int(create)
        counts = np.zeros(K_MAX, np.float32)
        np.add.at(counts, z, 1.0)
        sums = np.zeros((K_MAX, d), np.float32)
        np.add.at(sums, z, X)
        mu = np.where((counts > 0)[:, None],
                      sums / np.maximum(counts, 1.0)[:, None], mu_c)
        obj = float(dvals.sum()) + LAMBDA * K
        if it > 0 and abs(obj - prev_obj) < OBJ_TOL * obj:
            break
        prev_obj = obj
    return mu


def run_device(X: np.ndarray, ncores: int = NCORES):
    n, d = X.shape
    assert d == D and n % P == 0
    nt = n // P
    x2 = np.einsum('ij,ij->i', X.astype(np.float64), X.astype(np.float64)) \
        .astype(np.float32)
    x2pm = np.ascontiguousarray(x2.reshape(nt, P).T)
    nc_ = _get_nc(n, ncores)
    in_maps = []
    for c in range(ncores):
        m = prep_core(X, c)
        m["x2pm"] = x2pm
        in_maps.append(m)
    res = bass_utils.run_bass_kernel_spmd(
        nc_, in_maps, core_ids=list(range(ncores)))
    outs = [r["o_out"][0].astype(np.float64) for r in res.results]
    colsum = np.concatenate([o[0:P] for o in outs])
    S_far = np.concatenate([o[P:2 * P] for o in outs])
    sums = np.concatenate(
        [o[2 * P:5 * P].reshape(P, 3).T for o in outs], axis=1)
    sc = outs[0][5 * P:5 * P + NSC]
    nfar0, cnt0, cnt1, cnt2, snfdmin, sfx2_0, sfx2_1 = sc[:7]
    cnts = np.array([cnt0, cnt1, cnt2])
    nn0 = n - nfar0
    if not (nfar0 > 0.5 and nn0 > 0.5 and cnts.min() > 0.5):
        return None
    sx2 = float(x2.astype(np.float64).sum())
    mu0 = colsum / n
    S_near = colsum - S_far
    c1 = S_far / nfar0
    sum_near_d0 = (sx2 - sfx2_0) - 2.0 * float(S_near @ mu0) + nn0 * float(mu0 @ mu0)
    sum_far_d0 = sfx2_0 - 2.0 * float(S_far @ c1) + nfar0 * float(c1 @ c1)
    obj0 = sum_near_d0 + sum_far_d0 + LAMBDA * 2.0
    sum_far_d1 = sfx2_1 - float(sums[2] @ sums[2]) / cnt2
    obj1 = snfdmin + sum_far_d1 + LAMBDA * 3.0
    converged = abs(obj1 - obj0) < OBJ_TOL * obj1
    margin = abs(obj1 - obj0) / (OBJ_TOL * obj1)
    if not converged or margin > 0.5:
        return None
    mu = np.zeros((K_MAX, D), np.float32)
    mu[0:3] = (sums / cnts[:, None]).astype(np.float32)
    return mu, dict(nfar0=nfar0, cnts=cnts, obj0=obj0, obj1=obj1, margin=margin)


def kernel(x: np.ndarray) -> np.ndarray:
    X = np.asarray(x[0], dtype=np.float32)
    out = run_device(X)
    if out is None:
        mu = _dpmeans_numpy_fallback(X)
    else:
        mu, _ = out
    return mu[None, :, :]


if __name__ == "__main__":
    nc_ = build_kernel(N_FULL, 8, reps=1)
    print("built ok")



# revision 2
# speedup vs baseline: 1.0247x; 1.0247x over previous
"""DP-means (2-iteration early-stop) TRN2 kernel v3 — fp16, flipped matmuls,
chunked AllReduces overlapped with masked-sum passes.

Same math as v2 (see kernel_v2.py docstring). v3 splits each AllReduce into
two point-range halves (cm partitions 0:64 / 64:128) so that far0/S_far (and
the it1 masks/segsums) for the first half run while the second half's
AllReduce is still in flight — hiding collective latency and keeping the PE
warm. colsum is split DVE/ACT. dist pipelines are computed in place.
"""
import sys

sys.path.insert(0, "/opt/trn_rl_repo")

import numpy as np

import concourse.bass as bass
import concourse.bacc as bacc
import concourse.tile as tile
from concourse import mybir
from concourse import bass_utils
from concourse.masks import make_identity

F32 = mybir.dt.float32
F16 = mybir.dt.float16
F8 = mybir.dt.float8e4
Alu = mybir.AluOpType
Act = mybir.ActivationFunctionType

N_FULL, D, K_MAX = 65536, 1024, 64
NCORES = 8
LAMBDA = 1000.0
MAX_ITER = 50
OBJ_TOL = 1e-3
P = 128
HP = 64
FC = D // NCORES
NSC = 8


def build_kernel(n_total: int = N_FULL, ncores: int = NCORES, reps: int = 1,
                 upto: int = 9, fake_coll: bool = False, serialize: bool = False,
                 ring_bufs: int = 5):
    NS = n_total
    NCM = NS // P            # cm cols (512)
    NCH = NS // 512          # moving chunks of 512 (128)
    HCH = NCH // 2
    S2 = NS // 2
    NRING = NS // 4096       # ring tiles of [P, 4096] (16)
    QB = 4

    nc = bacc.Bacc("TRN2", target_bir_lowering=False, debug=False,
                   num_devices=ncores)
    tr16 = nc.dram_tensor("tr16", [FC, NS], F16, kind="ExternalInput")
    nat16 = nc.dram_tensor("nat16", [P, NS], F16, kind="ExternalInput")
    x2cm_in = nc.dram_tensor("x2cm", [P, NCM], F32, kind="ExternalInput")
    o_out = nc.dram_tensor("o_out", [1, 5 * P + NSC], F32, kind="ExternalOutput")
    o_dep = nc.dram_tensor("o_dep", [1, 1], F16, kind="ExternalOutput") \
        if serialize else None

    rg = [list(range(ncores))]

    with tile.TileContext(nc) as tc:
        with (
            tc.tile_pool(name="persist", bufs=1) as pp,
            tc.tile_pool(name="ring", bufs=ring_bufs) as rp,
            tc.tile_pool(name="scratch", bufs=1) as scp,
            tc.tile_pool(name="psum", bufs=1, space="PSUM") as psp,
            tc.tile_pool(name="dram", bufs=1, space="DRAM") as dp,
        ):
            def _body(rep):
                onesf = pp.tile([P, 1], F32)
                nc.gpsimd.memset(onesf[:], 1.0)
                ident16 = pp.tile([P, P], F16)
                make_identity(nc, ident16[:])
                ident32 = pp.tile([P, P], F32)
                make_identity(nc, ident32[:])

                sdram = dp.tile([1, 16], F32)

                def psum11(col_f32, stag):
                    ps = psp.tile([1, 1], F32, tag="t11", bufs=1)
                    nc.tensor.matmul(ps[:], col_f32[:], onesf[:], start=True,
                                     stop=True)
                    s11 = scp.tile([1, 1], F32, tag="s11" + stag)
                    nc.vector.tensor_copy(s11[:], ps[:])
                    return s11

                def bcast(dst, buf, off):
                    src = buf[:, off:off + 1].rearrange("o u -> (o u)")
                    rep_ap = bass.AP(src.tensor, src.offset, [[0, P]] + src.ap)
                    nc.scalar.dma_start(dst[:], rep_ap)

                # ---------- resident tr16 (16MB) ----------
                tr_sb = pp.tile([P, NS], F16)
                if serialize:
                    dep16 = scp.tile([1, 1], F16, tag="dep")
                    nc.scalar.dma_start(dep16[:], o_dep[:, 0:1])
                    nc.scalar.dma_start(tr_sb[0:1, 0:1], dep16[:])
                for q in range(8):
                    nc.sync.dma_start(tr_sb[:, q * (NS // 8):(q + 1) * (NS // 8)],
                                      tr16[:, q * (NS // 8):(q + 1) * (NS // 8)])
                x2cm = pp.tile([P, NCM], F32)
                nc.scalar.dma_start(x2cm[:], x2cm_in[:, :])

                # persistent working tiles
                d0cm = pp.tile([P, NCM], F32)
                dScm = pp.tile([P, NCM], F32)
                dist0p = scp.tile([P, NCM], F32, tag="d0p")
                dist1 = scp.tile([P, NCM], F32, tag="d1")
                dmin = scp.tile([P, NCM], F32, tag="dmin")
                far0cm = pp.tile([P, NCM], F16)
                z1cm = pp.tile([P, NCM], F16)
                far1cm = pp.tile([P, NCM], F16)
                nf1cm = pp.tile([P, NCM], F16)
                a0cm = pp.tile([P, NCM], F16)
                a1cm = pp.tile([P, NCM], F16)
                far0T = pp.tile([P, NCM], F16)
                a3T = pp.tile([P, 3 * NCM], F16)
                Tmu = pp.tile([P, 2 * P + 1], F16)
                TS = pp.tile([P, 2 * P + 1], F16)
                colsum = pp.tile([P, 1], F32)
                cs16 = pp.tile([P, 16], F32)
                mu0 = pp.tile([P, 1], F32)
                scol = pp.tile([P, 1], F32)
                m02b = pp.tile([P, 1], F32)
                s2b = pp.tile([P, 1], F32)
                csb = pp.tile([P, 1], F32)
                rnfb = pp.tile([P, 1], F32)
                rnnb = pp.tile([P, 1], F32)
                nfar_p = pp.tile([P, 1], F32)
                sums3 = pp.tile([P, 3], F32)
                pack8 = pp.tile([P, NSC], F32)
                dst_st = pp.tile([P, NCM], F32)      # shared d0/dS staging
                cst = scp.tile([P, 512], F32, tag="cst")  # sf/seg psum staging
                fstg = scp.tile([HP, NCM], F16, tag="fstg")  # base-0 staging
                dump8 = scp.tile([P, 4096], F8, tag="dump8")

                b1i = [dp.tile([1, S2 + 8], F32, tag="b1i%d" % h, name="b1i%d" % h) for h in (0, 1)]
                b1o = [dp.tile([1, S2 + 8], F32, tag="b1o%d" % h, name="b1o%d" % h) for h in (0, 1)]
                b2i = [dp.tile([1, S2 + 8], F32, tag="b2i%d" % h, name="b2i%d" % h) for h in (0, 1)]
                b2o = [dp.tile([1, S2 + 8], F32, tag="b2o%d" % h, name="b2o%d" % h) for h in (0, 1)]

                # ---------- P1: colsum (12 DVE chunks + 4 ACT chunks) ----------
                for q in range(16):
                    sl = slice(q * (NS // 16), (q + 1) * (NS // 16))
                    if q < 12:
                        nc.vector.tensor_reduce(cs16[:, q:q + 1], tr_sb[:, sl],
                                                axis=mybir.AxisListType.X,
                                                op=Alu.add)
                    else:
                        nc.scalar.activation(dump8[:], tr_sb[:, sl], Act.Copy,
                                             accum_out=cs16[:, q:q + 1])
                nc.vector.tensor_reduce(colsum[:], cs16[:],
                                        axis=mybir.AxisListType.X, op=Alu.add)
                nc.vector.tensor_scalar(mu0[:], colsum[:], 1.0 / n_total, None,
                                        op0=Alu.mult)
                nc.gpsimd.memset(Tmu[:], 0.0)
                nc.vector.tensor_copy(Tmu[:, P:P + 1], mu0[:])
                mu0q = scp.tile([P, 1], F32, tag="mu0q")
                nc.vector.tensor_copy(mu0q[:], Tmu[:, P:P + 1])
                sq = scp.tile([P, 1], F32, tag="sq")
                nc.vector.tensor_tensor(sq[:], mu0q[:], mu0q[:], op=Alu.mult)
                m02s = psum11(sq, 'm02')
                nc.scalar.dma_start(b1i[0][:, S2:S2 + 1], m02s[:])

                if upto < 2:
                    return

                # ---------- P2: d0 halves + AR1a/AR1b ----------
                def dot_half(Tst, bout, hh, tag):
                    ps = psp.tile([P, 512], F32, tag="dot", bufs=2)
                    for cc in range(HCH):
                        c = hh * HCH + cc
                        nc.tensor.matmul(ps[:], Tst[:, P - c:2 * P - c],
                                         tr_sb[:, c * 512:(c + 1) * 512],
                                         start=(cc == 0), stop=(cc == HCH - 1))
                    rows = slice(hh * HP, (hh + 1) * HP)
                    nc.vector.tensor_copy(dst_st[rows, :], ps[rows, :])
                    nc.scalar.dma_start(
                        bout[:, 0:S2].rearrange("o (p j) -> (o p) j", j=NCM),
                        dst_st[rows, :])

                for hh in (0, 1):
                    dot_half(Tmu, b1i[hh], hh, "d0")
                    if fake_coll:
                        nc.sync.dma_start(b1o[hh][:], b1i[hh][:])
                    else:
                        nc.gpsimd.collective_compute(
                            "AllReduce", Alu.add, replica_groups=rg,
                            ins=[b1i[hh].opt()], outs=[b1o[hh].opt()])
                if upto < 3:
                    return

                # ---------- P3/P4: far0 + transposes + S_far per half ----------
                bcast(m02b, b1o[0], S2)
                far0T_r = far0T[:].rearrange("q (m c) -> q m c", m=QB)
                sf_ps = psp.tile([QB, 512], F32, tag="sf")

                def far_half(hh):
                    rows = slice(hh * HP, (hh + 1) * HP)
                    nc.scalar.dma_start(
                        d0cm[rows, :],
                        b1o[hh][:, 0:S2].rearrange("o (p j) -> (o p) j", j=NCM))
                    nc.vector.tensor_scalar(dist0p[rows, :], d0cm[rows, :], -2.0,
                                            None, op0=Alu.mult)
                    nc.vector.tensor_tensor(dist0p[rows, :], dist0p[rows, :],
                                            x2cm[rows, :], op=Alu.add)
                    nc.vector.tensor_scalar(dist0p[rows, :], dist0p[rows, :],
                                            m02b[rows, :], None, op0=Alu.add)
                    nc.vector.tensor_scalar(far0cm[rows, :], dist0p[rows, :],
                                            LAMBDA, None, op0=Alu.is_gt)
                    nc.vector.tensor_reduce(nfar_p[rows, :], far0cm[rows, :],
                                            axis=mybir.AxisListType.X, op=Alu.add)
                    src = far0cm
                    roff = hh * HP
                    if hh == 1:
                        nc.scalar.dma_start(fstg[:, :], far0cm[rows, :])
                        src, roff = fstg, 0
                    for m in range(QB):
                        trp = psp.tile([P, P], F16, tag="trp", bufs=1)
                        nc.tensor.transpose(
                            trp[:, 0:HP],
                            src[roff:roff + HP, m * P:(m + 1) * P],
                            ident16[0:HP, 0:HP])
                        nc.vector.tensor_copy(
                            far0T[:, m * P + hh * HP:m * P + (hh + 1) * HP],
                            trp[:, 0:HP])

                def sf_half(hh):
                    for rr in range(NRING // 2):
                        r = hh * (NRING // 2) + rr
                        natg = rp.tile([P, 4096], F16, tag="nat")
                        nc.sync.dma_start(natg[:],
                                          nat16[:, r * 4096:(r + 1) * 4096])
                        for cc in range(8):
                            c = r * 8 + cc
                            nc.tensor.matmul(
                                sf_ps[:], far0T_r[:, :, c:c + 1],
                                natg[:, cc * 512:(cc + 1) * 512],
                                start=(c == 0), stop=(c == NCH - 1),
                                skip_group_check=(c != 0))

                for hh in (0, 1):
                    far_half(hh)
                    if upto < 4:
                        return
                    sf_half(hh)

                # nfar scalars (after far0 both halves)
                nf11 = psum11(nfar_p, 'nf')
                rnf11 = scp.tile([1, 1], F32, tag="rnf")
                nc.vector.reciprocal(rnf11[:], nf11[:])
                nn11 = scp.tile([1, 1], F32, tag="nn")
                nc.vector.tensor_scalar(nn11[:], nf11[:], -1.0, float(n_total),
                                        op0=Alu.mult, op1=Alu.add)
                rnn11 = scp.tile([1, 1], F32, tag="rnn")
                nc.vector.reciprocal(rnn11[:], nn11[:])
                nc.scalar.dma_start(sdram[:, 0:1], rnf11[:])
                nc.scalar.dma_start(sdram[:, 1:2], rnn11[:])
                bcast(rnfb, sdram, 0)
                bcast(rnnb, sdram, 1)
                nc.vector.tensor_copy(pack8[:, 0:1], nfar_p[:])
                # sfx2_0
                nc.vector.tensor_copy(dist1[:], far0cm[:])
                nc.vector.tensor_tensor(dist1[:], dist1[:], x2cm[:], op=Alu.mult)
                nc.vector.tensor_reduce(pack8[:, 5:6], dist1[:],
                                        axis=mybir.AxisListType.X, op=Alu.add)
                # S_far combine -> scol -> TS
                nc.vector.tensor_copy(cst[0:QB, :], sf_ps[:])
                for m in range(QB):
                    trf = psp.tile([P, P], F32, tag="trf", bufs=1)
                    nc.tensor.transpose(trf[:], cst[:, m * P:(m + 1) * P],
                                        ident32[:])
                    if m == 0:
                        nc.vector.tensor_copy(scol[:], trf[:, 0:1])
                    else:
                        nc.vector.tensor_tensor(scol[:], scol[:], trf[:, m:m + 1],
                                                op=Alu.add)
                nc.gpsimd.memset(TS[:], 0.0)
                nc.vector.tensor_copy(TS[:, P:P + 1], scol[:])
                sq2 = scp.tile([P, 1], F32, tag="sq2")
                scolq = scp.tile([P, 1], F32, tag="scolq")
                nc.vector.tensor_copy(scolq[:], TS[:, P:P + 1])
                nc.vector.tensor_tensor(sq2[:], scolq[:], scolq[:], op=Alu.mult)
                s2s = psum11(sq2, 's2')
                nc.scalar.dma_start(b2i[0][:, S2:S2 + 1], s2s[:])
                nc.vector.tensor_tensor(sq2[:], colsum[:], scolq[:], op=Alu.mult)
                css = psum11(sq2, 'cs')
                nc.scalar.dma_start(b2i[0][:, S2 + 1:S2 + 2], css[:])
                if upto < 5:
                    return

                # ---------- P5: dS halves + AR2a/AR2b ----------
                for hh in (0, 1):
                    dot_half(TS, b2i[hh], hh, "dS")
                    if fake_coll:
                        nc.sync.dma_start(b2o[hh][:], b2i[hh][:])
                    else:
                        nc.gpsimd.collective_compute(
                            "AllReduce", Alu.add, replica_groups=rg,
                            ins=[b2i[hh].opt()], outs=[b2o[hh].opt()])
                if upto < 6:
                    return

                # ---------- P6/P7: it1 masks + segsums per half ----------
                bcast(s2b, b2o[0], S2)
                bcast(csb, b2o[0], S2 + 1)
                lane = scp.tile([P, 1], F32, tag="lane")
                lane2 = scp.tile([P, 1], F32, tag="lane2")
                c1sqb = scp.tile([P, 1], F32, tag="c1sq")
                m0psqb = scp.tile([P, 1], F32, tag="m0psq")
                a1b = scp.tile([P, 1], F32, tag="a1b")
                a2b = scp.tile([P, 1], F32, tag="a2b")
                nc.vector.tensor_tensor(lane[:], s2b[:], rnfb[:], op=Alu.mult)
                nc.vector.tensor_tensor(c1sqb[:], lane[:], rnfb[:], op=Alu.mult)
                nc.vector.tensor_scalar(a1b[:], rnfb[:], -2.0, None, op0=Alu.mult)
                nc.vector.tensor_scalar(lane[:], csb[:], -2.0, None, op0=Alu.mult)
                nc.vector.tensor_scalar(lane2[:], m02b[:],
                                        float(n_total) * float(n_total), lane[:],
                                        op0=Alu.mult, op1=Alu.add)
                nc.vector.tensor_tensor(lane2[:], lane2[:], s2b[:], op=Alu.add)
                nc.vector.tensor_tensor(lane2[:], lane2[:], rnnb[:], op=Alu.mult)
                nc.vector.tensor_tensor(m0psqb[:], lane2[:], rnnb[:], op=Alu.mult)
                nc.vector.tensor_scalar(a2b[:], rnnb[:], -2.0, None, op0=Alu.mult)

                a3T_r = a3T[:].rearrange("q (c m k) -> q c m k", m=QB, k=3)
                seg_ps = psp.tile([3 * QB, 512], F32, tag="seg")

                def mask_half(hh):
                    rows = slice(hh * HP, (hh + 1) * HP)
                    nc.scalar.dma_start(
                        dScm[rows, :],
                        b2o[hh][:, 0:S2].rearrange("o (p j) -> (o p) j", j=NCM))
                    # dist1 = dS*a1 + x2 + |c1|^2
                    nc.vector.tensor_scalar(dist1[rows, :], dScm[rows, :],
                                            a1b[rows, :], None, op0=Alu.mult)
                    nc.vector.tensor_tensor(dist1[rows, :], dist1[rows, :],
                                            x2cm[rows, :], op=Alu.add)
                    nc.vector.tensor_scalar(dist1[rows, :], dist1[rows, :],
                                            c1sqb[rows, :], None, op0=Alu.add)
                    # dist0p = (N*d0 - dS)*a2 + x2 + |mu0p|^2
                    nc.vector.tensor_scalar(dist0p[rows, :], d0cm[rows, :],
                                            float(n_total), None, op0=Alu.mult)
                    nc.vector.tensor_tensor(dist0p[rows, :], dist0p[rows, :],
                                            dScm[rows, :], op=Alu.subtract)
                    nc.vector.tensor_scalar(dist0p[rows, :], dist0p[rows, :],
                                            a2b[rows, :], None, op0=Alu.mult)
                    nc.vector.tensor_tensor(dist0p[rows, :], dist0p[rows, :],
                                            x2cm[rows, :], op=Alu.add)
                    nc.vector.tensor_scalar(dist0p[rows, :], dist0p[rows, :],
                                            m0psqb[rows, :], None, op0=Alu.add)
                    nc.vector.tensor_tensor(z1cm[rows, :], dist1[rows, :],
                                            dist0p[rows, :], op=Alu.is_lt)
                    nc.vector.tensor_tensor(dmin[rows, :], dist0p[rows, :],
                                            dist1[rows, :], op=Alu.min)
                    nc.vector.tensor_scalar(far1cm[rows, :], dmin[rows, :],
                                            LAMBDA, None, op0=Alu.is_gt)
                    nc.vector.tensor_scalar(nf1cm[rows, :], far1cm[rows, :],
                                            -1.0, 1.0, op0=Alu.mult, op1=Alu.add)
                    nc.vector.tensor_scalar(a0cm[rows, :], z1cm[rows, :],
                                            -1.0, 1.0, op0=Alu.mult, op1=Alu.add)
                    nc.vector.tensor_tensor(a0cm[rows, :], a0cm[rows, :],
                                            nf1cm[rows, :], op=Alu.mult)
                    nc.vector.tensor_tensor(a1cm[rows, :], z1cm[rows, :],
                                            nf1cm[rows, :], op=Alu.mult)
                    for k, mk in ((0, a0cm), (1, a1cm), (2, far1cm)):
                        src, roff = mk, hh * HP
                        if hh == 1:
                            nc.scalar.dma_start(fstg[:, :], mk[rows, :])
                            src, roff = fstg, 0
                        for m in range(QB):
                            trp = psp.tile([P, P], F16, tag="trp", bufs=1)
                            nc.tensor.transpose(
                                trp[:, 0:HP],
                                src[roff:roff + HP, m * P:(m + 1) * P],
                                ident16[0:HP, 0:HP])
                            nc.vector.tensor_copy(
                                a3T_r[:, hh * HP:(hh + 1) * HP, m, k],
                                trp[:, 0:HP])

                def seg_half(hh):
                    for rr in range(NRING // 2):
                        r = hh * (NRING // 2) + rr
                        natg = rp.tile([P, 4096], F16, tag="nat")
                        nc.sync.dma_start(natg[:],
                                          nat16[:, r * 4096:(r + 1) * 4096])
                        for cc in range(8):
                            c = r * 8 + cc
                            nc.tensor.matmul(
                                seg_ps[:], a3T[:, c * 12:(c + 1) * 12],
                                natg[:, cc * 512:(cc + 1) * 512],
                                start=(c == 0), stop=(c == NCH - 1),
                                skip_group_check=(c != 0))

                for hh in (0, 1):
                    mask_half(hh)
                    if upto < 7:
                        return
                    seg_half(hh)

                # ---------- stats + seg combine + outputs ----------
                nc.vector.tensor_reduce(pack8[:, 1:2], a0cm[:],
                                        axis=mybir.AxisListType.X, op=Alu.add)
                nc.vector.tensor_reduce(pack8[:, 2:3], a1cm[:],
                                        axis=mybir.AxisListType.X, op=Alu.add)
                nc.vector.tensor_reduce(pack8[:, 3:4], far1cm[:],
                                        axis=mybir.AxisListType.X, op=Alu.add)
                nc.vector.tensor_copy(dist1[:], nf1cm[:])
                nc.vector.tensor_tensor(dist1[:], dmin[:], dist1[:], op=Alu.mult)
                nc.vector.tensor_reduce(pack8[:, 4:5], dist1[:],
                                        axis=mybir.AxisListType.X, op=Alu.add)
                nc.vector.tensor_copy(dist1[:], far1cm[:])
                nc.vector.tensor_tensor(dist1[:], dist1[:], x2cm[:], op=Alu.mult)
                nc.vector.tensor_reduce(pack8[:, 6:7], dist1[:],
                                        axis=mybir.AxisListType.X, op=Alu.add)
                nc.vector.tensor_copy(pack8[:, 7:8], pack8[:, 6:7])

                nc.vector.tensor_copy(cst[0:3 * QB, :], seg_ps[:])
                for m in range(QB):
                    trf = psp.tile([P, P], F32, tag="trf", bufs=1)
                    nc.tensor.transpose(trf[:], cst[:, m * P:(m + 1) * P],
                                        ident32[:])
                    if m == 0:
                        nc.vector.tensor_copy(sums3[:], trf[:, 0:3])
                    else:
                        nc.vector.tensor_tensor(sums3[:], sums3[:],
                                                trf[:, 3 * m:3 * m + 3],
                                                op=Alu.add)
                nc.sync.dma_start(
                    o_out[:, 0:P].rearrange("o (p u) -> (o p) u", u=1), colsum[:])
                nc.sync.dma_start(
                    o_out[:, P:2 * P].rearrange("o (p u) -> (o p) u", u=1), scol[:])
                nc.sync.dma_start(
                    o_out[:, 2 * P:5 * P].rearrange("o (p k) -> (o p) k", k=3),
                    sums3[:])
                sc_ps = psp.tile([NSC, 1], F32, tag="t11")
                nc.tensor.matmul(sc_ps[:], pack8[:], onesf[:], start=True,
                                 stop=True)
                sc81 = scp.tile([NSC, 1], F32, tag="sc81")
                nc.vector.tensor_copy(sc81[:], sc_ps[:])
                nc.scalar.dma_start(
                    o_out[:, 5 * P:5 * P + NSC]
                    .rearrange("o (r u) -> (o r) u", u=1), sc81[:])
                if serialize:
                    dps = scp.tile([1, 1], F16, tag="deps")
                    nc.vector.tensor_copy(dps[:], far1cm[0:1, 0:1])
                    nc.scalar.dma_start(o_dep[:, 0:1], dps[:])

            for _rep in range(reps):
                _body(_rep)

    nc.compile()
    return nc


def prep_core(X, c):
    xs = np.ascontiguousarray(X[:, c * FC:(c + 1) * FC])
    xh = xs.astype(np.float16)
    n = xs.shape[0]
    tr = np.ascontiguousarray(xh.T)
    nat = np.ascontiguousarray(
        xh.reshape(n // P, P, FC).transpose(1, 0, 2).reshape(P, n))
    return {"tr16": tr, "nat16": nat}


_NC_CACHE = {}


def _get_nc(n_total, ncores):
    key = (n_total, ncores)
    if key not in _NC_CACHE:
        _NC_CACHE[key] = build_kernel(n_total, ncores)
    return _NC_CACHE[key]


def _dpmeans_numpy_fallback(X):
    n, d = X.shape
    mu = np.zeros((K_MAX, d), np.float32)
    mu[0] = X.mean(axis=0)
    K = 1
    x2 = np.sum(X * X, axis=1)
    prev_obj = 0.0
    for it in range(MAX_ITER):
        m2 = np.sum(mu * mu, axis=1)
        dist = x2[:, None] - 2.0 * (X @ mu.T) + m2[None, :]
        dist[:, K:] = 1e30
        dmin = dist.min(axis=1)
        z = dist.argmin(axis=1)
        far = dmin > LAMBDA
        create = bool(far.any()) and K < K_MAX
        Kc = min(K, K_MAX - 1)
        nfar = float(far.sum())
        new_center = (far.astype(np.float32) @ X) / max(nfar, 1.0)
        mu_c = mu.copy() if create else mu
        if create:
            mu_c[Kc] = new_center
            z = np.where(far, Kc, z)
            new_col = x2 - 2.0 * (X @ new_center) + float(new_center @ new_center)
            dvals = np.where(far, new_col, dmin)
        else:
            dvals = dmin
        K = K + int(create)
        counts = np.zeros(K_MAX, np.float32)
        np.add.at(counts, z, 1.0)
        sums = np.zeros((K_MAX, d), np.float32)
        np.add.at(sums, z, X)
        mu = np.where((counts > 0)[:, None],
                      sums / np.maximum(counts, 1.0)[:, None], mu_c)
        obj = float(dvals.sum()) + LAMBDA * K
        if it > 0 and abs(obj - prev_obj) < OBJ_TOL * obj:
            break
        prev_obj = obj
    return mu


def run_device(X: np.ndarray, ncores: int = NCORES):
    n, d = X.shape
    assert d == D and n % 512 == 0
    x2 = np.einsum('ij,ij->i', X.astype(np.float64), X.astype(np.float64)) \
        .astype(np.float32)
    x2cm = np.ascontiguousarray(x2.reshape(P, n // P))
    nc_ = _get_nc(n, ncores)
    in_maps = []
    for c in range(ncores):
        m = prep_core(X, c)
        m["x2cm"] = x2cm
        in_maps.append(m)
    res = bass_utils.run_bass_kernel_spmd(
        nc_, in_maps, core_ids=list(range(ncores)))
    outs = [r["o_out"][0].astype(np.float64) for r in res.results]
    colsum = np.concatenate([o[0:P] for o in outs])
    S_far = np.concatenate([o[P:2 * P] for o in outs])
    sums = np.concatenate(
        [o[2 * P:5 * P].reshape(P, 3).T for o in outs], axis=1)
    sc = outs[0][5 * P:5 * P + NSC]
    nfar0, cnt0, cnt1, cnt2, snfdmin, sfx2_0, sfx2_1 = sc[:7]
    cnts = np.array([cnt0, cnt1, cnt2])
    nn0 = n - nfar0
    if not (nfar0 > 0.5 and nn0 > 0.5 and cnts.min() > 0.5):
        return None
    sx2 = float(x2.astype(np.float64).sum())
    mu0 = colsum / n
    S_near = colsum - S_far
    c1 = S_far / nfar0
    sum_near_d0 = (sx2 - sfx2_0) - 2.0 * float(S_near @ mu0) + nn0 * float(mu0 @ mu0)
    sum_far_d0 = sfx2_0 - 2.0 * float(S_far @ c1) + nfar0 * float(c1 @ c1)
    obj0 = sum_near_d0 + sum_far_d0 + LAMBDA * 2.0
    sum_far_d1 = sfx2_1 - float(sums[2] @ sums[2]) / cnt2
    obj1 = snfdmin + sum_far_d1 + LAMBDA * 3.0
    converged = abs(obj1 - obj0) < OBJ_TOL * obj1
    margin = abs(obj1 - obj0) / (OBJ_TOL * obj1)
    if not converged or margin > 0.5:
        return None
    mu = np.zeros((K_MAX, D), np.float32)
    mu[0:3] = (sums / cnts[:, None]).astype(np.float32)
    return mu, dict(nfar0=nfar0, cnts=cnts, obj0=obj0, obj1=obj1, margin=margin)


def kernel(x: np.ndarray) -> np.ndarray:
    X = np.asarray(x[0], dtype=np.float32)
    out = run_device(X)
    if out is None:
        mu = _dpmeans_numpy_fallback(X)
    else:
        mu, _ = out
    return mu[None, :, :]


if __name__ == "__main__":
    nc_ = build_kernel(N_FULL, 8, reps=1)
    print("built ok")
